# revision 1
# baseline (speedup 1.0000x reference)
"""Trainium2 Bass kernel for nn_CruxMiniCircuit (gnn_message_passing).

Reference semantics: B independent rows; each row is a circuit of N nodes
(literal nodes hold a fixed one-hot distribution over 10 ints, op nodes
combine left/right child distributions through a per-op bilinear table
followed by softmax).  The reference runs 10 synchronous passes over all
nodes and returns only the root (node 0) logits per row.

Key observation: the output depends only on node 0's dependency cone
unrolled 10 passes deep.  Literal children are compile-time constants
(one-hot vectors) and op nodes at pass 0 are zero, so the per-row
worklists are tiny (mean ~5 updates/row for the benchmark distribution).
The host precomputes integer worklists / gather indices; the device does
all floating-point math.

Device pipeline per pass: ap_gather pulls child value vectors out of
per-row-group SBUF value buffers (rows are binned into 8 groups so all 8
GPSIMD Q7 cores gather concurrently); small DMAs concatenate the
group-blocked gather output into contraction layout; TensorE builds the
replicated operands and the bilinear contraction; softmax runs as
exp (ScalarE) + ones-matmul reduction (TensorE) + reciprocal (VectorE);
all three per-op softmax results are stored so op selection folds into
the next pass's gather indices.  Pass-1 inputs are constants and are
shipped from the host directly, skipping one gather.

Sharding: pure data parallel over the batch dim (B=2048 -> 256 rows on
each of the 8 NeuronCores), per the sharding hint.  No collectives are
needed for the forward pass.
"""

import sys
from contextlib import ExitStack

import numpy as np

sys.path.insert(0, "/opt/trn_rl_repo")

import concourse.bass as bass
import concourse.tile as tile
from concourse import bacc, mybir
from concourse.bass_utils import run_bass_kernel_spmd

B, N = 2048, 1023
NI, NO, NP = 10, 3, 10  # n_ints, n_ops, n_passes
NCORES = 8
RPC = B // NCORES  # rows per core
NG = 8  # row groups per core (one per Q7 core / 16-partition block)
ZSLOT = NI  # value-buffer slot holding the zero vector
NCONST = NI + 1  # slots 0..9 = one-hot e_k, slot 10 = zeros
CHUNK = 448  # free-dim chunk for the compute pipeline (PSUM/matmul limits)

TRACE = False  # set True (e.g. from test.py) to profile the HW run
LAST_RESULTS = None  # BassKernelResults of the last run (exec_time_ns etc.)


def _plan(cats, ops, lits, left, right, mask):
    """Integer-only preprocessing: worklists, group binning, gather indices."""
    left = np.clip(left.astype(np.int64), 0, N - 1)
    right = np.clip(right.astype(np.int64), 0, N - 1)
    opsc = np.clip(ops.astype(np.int64), 0, NO - 1)
    litsc = np.clip(lits.astype(np.int64), 0, NI - 1)
    m = mask.astype(bool)
    is_lit = (cats == 0) & m
    is_opa = (cats == 1) & m
    const_slot = np.where(is_lit, litsc, ZSLOT)

    # Worklists W[p]: (row, node) updates needed at pass p.
    Wr = [None] * (NP + 1)
    Wn = [None] * (NP + 1)
    r10 = np.nonzero(cats[:, 0] == 1)[0].astype(np.int64)
    Wr[NP], Wn[NP] = r10, np.zeros(len(r10), np.int64)
    need = np.zeros((B, N), bool)
    for p in range(NP, 1, -1):
        r, n = Wr[p], Wn[p]
        cr = np.concatenate([r, r])
        cn = np.concatenate([left[r, n], right[r, n]])
        keep = is_opa[cr, cn]
        need[:] = False
        need[cr[keep], cn[keep]] = True
        rr, nn = np.nonzero(need)
        Wr[p - 1], Wn[p - 1] = rr.astype(np.int64), nn.astype(np.int64)

    # Bin rows into NG groups per core, balancing total updates per group.
    weight = np.zeros(B, np.int64)
    for p in range(1, NP + 1):
        np.add.at(weight, Wr[p], 1)
    G = np.zeros(B, np.int64)
    for c in range(NCORES):
        rows = np.arange(c * RPC, (c + 1) * RPC)
        order = rows[np.argsort(-weight[rows], kind="stable")]
        load = np.zeros(NG, np.int64)
        for rr_ in order:
            g = int(load.argmin())
            G[rr_] = g
            load[g] += weight[rr_]

    # Per-pass group-local ids and padded per-group size Q_p.
    Qp = np.zeros(NP + 1, np.int64)
    gid = [None] * (NP + 1)
    for p in range(1, NP + 1):
        r = Wr[p]
        core = r // RPC
        grp = G[r]
        key = core * NG + grp
        order = np.argsort(key, kind="stable")
        ks = key[order]
        u = np.arange(len(ks), dtype=np.int64)
        if len(ks):
            first = np.r_[True, ks[1:] != ks[:-1]]
            seg_idx = np.nonzero(first)[0]
            u = u - seg_idx[np.cumsum(first) - 1]
        ul = np.empty(len(ks), np.int64)
        ul[order] = u
        cnt = np.bincount(key, minlength=NCORES * NG) if len(r) else np.zeros(NCORES * NG, np.int64)
        mx = int(cnt.max()) if len(r) else 0
        Qp[p] = max(8, -(-mx // 8) * 8)  # multiple of 8 -> num_idxs % 16 == 0
        gid[p] = (core, grp, ul)

    # Buffer slot bases (group-local numbering); passes 1..NP-1 store 3 slots/update.
    base = np.zeros(NP + 1, np.int64)
    base[1] = NCONST
    for p in range(2, NP + 1):
        base[p] = base[p - 1] + 3 * Qp[p - 1]
    S = int(base[NP - 1] + 3 * Qp[NP - 1])
    assert S <= 32000, f"value buffer too large for int16 gather indices: {S}"

    idx_wrapped = []
    Ftot = 0
    slot_prev = np.full((B, N), -1, np.int64)
    lr1 = None
    for p in range(1, NP + 1):
        r, n = Wr[p], Wn[p]
        core, grp, ul = gid[p]
        lch, rch = left[r, n], right[r, n]
        if p == 1:
            lidx = const_slot[r, lch]
            ridx = const_slot[r, rch]
        else:
            lidx = np.where(is_opa[r, lch],
                            base[p - 1] + 3 * slot_prev[r, lch] + opsc[r, lch],
                            const_slot[r, lch])
            ridx = np.where(is_opa[r, rch],
                            base[p - 1] + 3 * slot_prev[r, rch] + opsc[r, rch],
                            const_slot[r, rch])
        Q = int(Qp[p])
        arr = np.full((NCORES, NG, 2 * Q), ZSLOT, np.int64)
        arr[core, grp, ul] = lidx
        arr[core, grp, Q + ul] = ridx
        if p == 1:
            # pass-1 inputs are constants; ship lr1 from host (skip the gather).
            # lr10 layout: (10, 2*NG*Q): l half col g*Q+u ; r half col NG*Q+g*Q+u
            eyeext = np.concatenate([np.eye(NI, dtype=np.float32),
                                     np.zeros((NI, 1), np.float32)], axis=1)
            cols = arr.reshape(NCORES, NG, 2, Q).transpose(0, 2, 1, 3).reshape(NCORES, 2 * NG * Q)
            lr1 = np.ascontiguousarray(eyeext[:, cols].transpose(1, 0, 2))  # (NCORES, 10, 2*NG*Q)
        else:
            F = -(-2 * Q // 16)
            F += F & 1  # 4-byte-aligned idx slices (ucode reads dwords)
            tmp = np.full((NCORES, NG, F * 16), ZSLOT, np.int64)
            tmp[:, :, : 2 * Q] = arr
            w = tmp.reshape(NCORES, NG, F, 16).transpose(0, 1, 3, 2).reshape(NCORES, NG * 16, F)
            idx_wrapped.append(w.astype(np.int16))
            Ftot += F
        if p < NP:
            slot_prev = np.full((B, N), -1, np.int64)
            slot_prev[r, n] = ul

    idx_full = np.concatenate(idx_wrapped, axis=2)  # (NCORES, 128, Ftot)

    return dict(
        Qp=Qp, base=base, S=S, idx=idx_full, Ftot=Ftot, lr1=lr1,
        r10=r10, gid10=gid[NP],
        opsc=opsc, litsc=litsc, is_lit=is_lit, m=m, G=G,
    )


_CUR_BASE = None


def _build_nc(S, Qp, Ftot):
    f32 = mybir.dt.float32
    Q10 = int(Qp[NP])
    PT10 = NG * Q10
    nc = bacc.Bacc(None)
    consts = nc.dram_tensor("consts", [NI, NCONST], f32, kind="ExternalInput")
    wmat = nc.dram_tensor("wmat", [100, 74], f32, kind="ExternalInput")
    repl = nc.dram_tensor("repl", [NI, 100], f32, kind="ExternalInput")
    reprm = nc.dram_tensor("reprm", [NI, 100], f32, kind="ExternalInput")
    oblk = nc.dram_tensor("oblk", [74, NO], f32, kind="ExternalInput")
    oblk2 = nc.dram_tensor("oblk2", [NO, 74], f32, kind="ExternalInput")
    idx_in = nc.dram_tensor("idx", [128, Ftot], mybir.dt.int16, kind="ExternalInput")
    PT1 = NG * int(Qp[1])
    lr1_in = nc.dram_tensor("lr1", [NI, 2 * PT1], f32, kind="ExternalInput")
    outz = nc.dram_tensor("outz", [74, PT10], f32, kind="ExternalOutput")

    with ExitStack() as ctx:
        tc = ctx.enter_context(tile.TileContext(nc))
        singles = ctx.enter_context(tc.tile_pool(name="singles", bufs=1))
        work = ctx.enter_context(tc.tile_pool(name="work", bufs=2))
        psum = ctx.enter_context(tc.tile_pool(name="psum", bufs=1, space="PSUM"))
        lrpool = ctx.enter_context(tc.tile_pool(name="lrpool", bufs=1))

        buf = singles.tile([128, S], f32)
        nc.vector.memset(buf[:, :], 0.0)
        for g in range(NG):
            nc.sync.dma_start(out=buf[16 * g : 16 * g + NI, 0:NCONST], in_=consts[:, :])
        w_sb = singles.tile([100, 74], f32)
        nc.sync.dma_start(out=w_sb[:, :], in_=wmat[:, :])
        repl_sb = singles.tile([NI, 100], f32)
        nc.sync.dma_start(out=repl_sb[:, :], in_=repl[:, :])
        reprm_sb = singles.tile([NI, 100], f32)
        nc.sync.dma_start(out=reprm_sb[:, :], in_=reprm[:, :])
        oblk_sb = singles.tile([74, NO], f32)
        nc.sync.dma_start(out=oblk_sb[:, :], in_=oblk[:, :])
        oblk2_sb = singles.tile([NO, 74], f32)
        nc.sync.dma_start(out=oblk2_sb[:, :], in_=oblk2[:, :])
        idx_sb = singles.tile([128, Ftot], mybir.dt.int16)
        nc.sync.dma_start(out=idx_sb[:, :], in_=idx_in[:, :])

        foff = 0
        for p in range(1, NP + 1):
            Q = int(Qp[p])
            PT = NG * Q
            lr10 = lrpool.tile([NI, 2 * PT], f32, tag=f"lr10_{p}")
            if p == 1:
                nc.sync.dma_start(out=lr10[:, :], in_=lr1_in[:, :])
            else:
                F = -(-2 * Q // 16)
                F += F & 1
                lrg = lrpool.tile([128, 2 * Q], f32, tag=f"lrg{p}")
                nc.gpsimd.ap_gather(
                    out_ap=lrg[:, :],
                    in_ap=buf[:, :],
                    idxs_ap=idx_sb[:, foff : foff + F],
                    channels=128,
                    num_elems=S,
                    d=1,
                    num_idxs=2 * Q,
                )
                foff += F
                # concat groups: lr10[i, h*PT + g*Q + u] = lrg[16g+i, h*Q + u]
                for g in range(NG):
                    src = lrg[16 * g : 16 * g + NI, :].rearrange("i (h u) -> i h u", h=2)
                    dst = lr10[:, :].rearrange("i (h gg u) -> i h gg u", h=2, gg=NG)[:, :, g, :]
                    nc.sync.dma_start(out=dst, in_=src)
            for c0 in range(0, PT, CHUNK):
                cw = min(CHUNK, PT - c0)
                ps_l = psum.tile([100, cw], f32, tag="ps_l")
                nc.tensor.matmul(ps_l[:, :], repl_sb[:, :], lr10[:, c0 : c0 + cw],
                                 start=True, stop=True)
                ps_r = psum.tile([100, cw], f32, tag="ps_r")
                nc.tensor.matmul(ps_r[:, :], reprm_sb[:, :], lr10[:, PT + c0 : PT + c0 + cw],
                                 start=True, stop=True)
                lrep_sb = work.tile([100, cw], f32, tag="lrep_sb")
                nc.vector.tensor_copy(lrep_sb[:, :], ps_l[:, :])
                outer = work.tile([100, cw], f32, tag="outer")
                nc.vector.tensor_mul(outer[:, :], lrep_sb[:, :], ps_r[:, :])
                ps_z = psum.tile([74, cw], f32, tag="ps_z")
                nc.tensor.matmul(ps_z[:, :], w_sb[:, :], outer[:, :], start=True, stop=True)
                if p == NP:
                    zsb = work.tile([74, cw], f32, tag="zsb")
                    nc.scalar.copy(zsb[:, :], ps_z[:, :])
                    nc.sync.dma_start(out=outz[:, c0 : c0 + cw], in_=zsb[:, :])
                    continue
                e = work.tile([74, cw], f32, tag="e")
                nc.scalar.activation(e[:, :], ps_z[:, :], mybir.ActivationFunctionType.Exp)
                ps_z3 = psum.tile([NO, cw], f32, tag="ps_z3")
                nc.tensor.matmul(ps_z3[:, :], oblk_sb[:, :], e[:, :], start=True, stop=True)
                rz = work.tile([NO, cw], f32, tag="rz")
                nc.vector.reciprocal(rz[:, :], ps_z3[:, :])
                ps_rz = psum.tile([74, cw], f32, tag="ps_rz")
                nc.tensor.matmul(ps_rz[:, :], oblk2_sb[:, :], rz[:, :], start=True, stop=True)
                st = work.tile([NI, 3 * cw], f32, tag="st")
                for o in range(NO):
                    nc.vector.tensor_mul(
                        st[:, o : 3 * cw : 3],
                        e[o * 32 : o * 32 + NI, :],
                        ps_rz[o * 32 : o * 32 + NI, :],
                    )
                # scatter back: buf[16g+k, b0+3u+o] = st[k, 3*(g*Q+u)+o]
                b0 = int(_CUR_BASE[p])
                for g in range(NG):
                    nc.sync.dma_start(
                        out=buf[16 * g : 16 * g + NI, b0 : b0 + 3 * Q],
                        in_=st[:, 3 * g * Q : 3 * (g + 1) * Q],
                    )
    nc.finalize()
    return nc


def kernel(op_table, cats, ops, lits, left, right, mask):
    global _CUR_BASE, LAST_RESULTS
    op_table = np.asarray(op_table, np.float32)
    plan = _plan(np.asarray(cats), np.asarray(ops), np.asarray(lits),
                 np.asarray(left), np.asarray(right), np.asarray(mask))
    Qp, base, S, Ftot = plan["Qp"], plan["base"], plan["S"], plan["Ftot"]
    _CUR_BASE = base
    assert NG * int(max(Qp[1:])) <= CHUNK, f"chunking not supported: {Qp}"

    nc = _build_nc(S, Qp, Ftot)

    consts = np.concatenate([np.eye(NI, dtype=np.float32),
                             np.zeros((NI, 1), np.float32)], axis=1)
    wmat = np.zeros((100, 74), np.float32)
    w30 = op_table.transpose(1, 2, 0, 3).reshape(100, 30)
    oblk = np.zeros((74, NO), np.float32)
    oblk2 = np.zeros((NO, 74), np.float32)
    for o in range(NO):
        wmat[:, o * 32 : o * 32 + NI] = w30[:, o * NI : (o + 1) * NI]
        oblk[o * 32 : o * 32 + NI, o] = 1.0
        oblk2[o, o * 32 : o * 32 + NI] = 1.0
    repl = np.kron(np.eye(NI), np.ones((1, NI))).astype(np.float32)
    reprm = np.tile(np.eye(NI), (1, NI)).astype(np.float32)

    in_maps = []
    for c in range(NCORES):
        in_maps.append({
            "consts": consts, "wmat": wmat, "repl": repl, "reprm": reprm,
            "oblk": oblk, "oblk2": oblk2,
            "idx": np.ascontiguousarray(plan["idx"][c]),
            "lr1": np.ascontiguousarray(plan["lr1"][c]),
        })

    res = run_bass_kernel_spmd(nc, in_maps, list(range(NCORES)), trace=TRACE)
    LAST_RESULTS = res

    # Assemble the full (B, NI) output on the host (index selection only).
    out = np.zeros((B, NI), np.float32)
    litsc, is_lit = plan["litsc"], plan["is_lit"]
    lit_rows = np.nonzero(cats[:, 0] == 0)[0]
    lr_active = is_lit[lit_rows, 0]
    oh = 10.0 * np.eye(NI, dtype=np.float32)[litsc[lit_rows, 0]]
    out[lit_rows] = np.where(lr_active[:, None], oh, 0.0)

    r10, opsc = plan["r10"], plan["opsc"]
    core10, grp10, ul10 = plan["gid10"]
    Q10 = int(Qp[NP])
    cols = grp10 * Q10 + ul10
    for c in range(NCORES):
        z = np.asarray(res.results[c]["outz"])  # (74, PT10)
        selmask = core10 == c
        rows = r10[selmask]
        cc = cols[selmask]
        o = opsc[rows, 0]
        zc = z[:, cc]
        sel = np.stack([zc[i * 32 : i * 32 + NI, :] for i in range(NO)])
        out[rows] = sel[o, :, np.arange(len(rows))]
    return out



# revision 3
# speedup vs baseline: 5.5675x; 5.5675x over previous
"""Trainium2 Bass kernel for nn_CruxMiniCircuit (gnn_message_passing).

Reference semantics: B independent rows; each row is a circuit of N nodes
(literal nodes hold a fixed one-hot distribution over 10 ints, op nodes
combine left/right child distributions through a per-op bilinear table
followed by softmax).  The reference runs 10 synchronous passes over all
nodes and returns only the root (node 0) logits per row.

Only node 0's dependency cone matters, unrolled 10 passes deep; the
per-row worklists are tiny (~5 updates/row).  The host precomputes the
integer structure; the device does all floating-point math.

This version keeps the entire per-pass pipeline on compute engines with
NO DMA / GPSIMD on the critical path (the previous gather+repack design
spent ~12us/pass on serialized SBUF-to-SBUF DMAs):

  * The per-pass state lives TRANSPOSED in SBUF as matmul weights:
    st[u, i*10+j] = value_i(update u) (and a second tile with the
    j-replication), rows 118..128 hold the constant literal one-hots.
  * Child gather + replication = ONE fp32 matmul per side per pass with a
    host-built one-hot selection matrix as the moving operand
    (ps_l = st.T @ SelL builds the replicated left operand directly).
  * outer = ps_l * ps_r (one scalar-engine PSUM->SBUF copy + one DVE mul).
  * Updates are op-sorted into fixed PE-tile segments (offsets 0/32/64),
    so the bilinear contraction is 3 static matmuls writing z transposed
    ([updates, 10]) straight into one PSUM tile.
  * softmax: one ScalarE exp with accum_out (gives the row sums for
    free), one DVE reciprocal_approx_fast, and two DVE broadcast
    tensor_scalar_muls that write the next pass's two state tiles.

Two independent row-streams per core interleave to hide the remaining
cross-engine latency.  Sharding: pure data parallel; op-root rows are
greedily bin-packed onto 8 cores x 2 streams balancing per-(pass, op)
update counts; literal-root rows never touch the device.
"""

import sys
from contextlib import ExitStack

import numpy as np

sys.path.insert(0, "/opt/trn_rl_repo")

import concourse.bass as bass  # noqa: F401
import concourse.tile as tile
from concourse import bacc, mybir
from concourse.bass_utils import run_bass_kernel_spmd

B, N = 2048, 1023
NI, NO, NP = 10, 3, 10  # n_ints, n_ops, n_passes
NCORES = 8
NB = 16  # bins = cores x streams
CONST0 = 118  # state rows 118..128 hold the literal one-hot constants
SEG_OFF = (0, 32, 64)  # PE-tile-aligned op segment offsets within a pass
CAPS = (32, 32, 54)

TRACE = False
LAST_RESULTS = None


def _plan(cats, ops, lits, left, right, mask):
    """Integer-only host preprocessing: worklists, binning, Sel matrices."""
    left = np.clip(left.astype(np.int64), 0, N - 1)
    right = np.clip(right.astype(np.int64), 0, N - 1)
    opsc = np.clip(ops.astype(np.int64), 0, NO - 1)
    litsc = np.clip(lits.astype(np.int64), 0, NI - 1)
    m = mask.astype(bool)
    is_lit = (cats == 0) & m
    is_opa = (cats == 1) & m

    # Worklists W[p]: (row, node) updates needed at pass p.
    Wr = [None] * (NP + 1)
    Wn = [None] * (NP + 1)
    r10 = np.nonzero(cats[:, 0] == 1)[0].astype(np.int64)
    Wr[NP], Wn[NP] = r10, np.zeros(len(r10), np.int64)
    need = np.zeros((B, N), bool)
    for p in range(NP, 1, -1):
        r, n = Wr[p], Wn[p]
        cr = np.concatenate([r, r])
        cn = np.concatenate([left[r, n], right[r, n]])
        keep = is_opa[cr, cn]
        need[:] = False
        need[cr[keep], cn[keep]] = True
        rr, nn = np.nonzero(need)
        Wr[p - 1], Wn[p - 1] = rr.astype(np.int64), nn.astype(np.int64)

    # Greedy bin-packing of op-root rows onto NB bins balancing per-(pass,
    # op) update counts (each bin's per-pass op segments must fit the fixed
    # PE-tile caps).
    cnt = np.zeros((B, NP, NO), np.int64)
    for p in range(1, NP + 1):
        np.add.at(cnt, (Wr[p], p - 1, opsc[Wr[p], Wn[p]]), 1)
    tot = cnt[r10].sum(axis=(1, 2))
    order = r10[np.argsort(-tot, kind="stable")]
    bins = np.zeros((NB, NP, NO), np.int64)
    binof = np.full(B, -1, np.int64)
    nrows = np.zeros(NB, np.int64)
    for r in order:
        c = cnt[r]
        best, bestscore = -1, None
        for b in range(NB):
            new = bins[b] + c
            score = (np.sort(new.reshape(-1))[::-1].tolist(), int(nrows[b]))
            if bestscore is None or score < bestscore:
                best, bestscore = b, score
        bins[best] += c
        binof[r] = best
        nrows[best] += 1

    # Per-pass op->segment permutation: the op with the largest max-count
    # takes the wide segment (cap 54); caps checked per pass.
    segop = np.zeros((NP, NO), np.int64)  # segop[p-1, seg] = op
    capsz = np.zeros((NP, NO), np.int64)  # actual segment sizes used
    Cp = np.zeros(NP + 1, np.int64)
    for p in range(1, NP + 1):
        mx = bins[:, p - 1, :].max(axis=0)  # per-op max over bins
        o_ord = np.argsort(mx, kind="stable")  # ascending
        segop[p - 1] = [o_ord[0], o_ord[1], o_ord[2]]
        for s in range(NO):
            assert mx[segop[p - 1, s]] <= CAPS[s], (p, s, mx)
        capsz[p - 1] = [CAPS[0], CAPS[1], mx[o_ord[2]]]
        Cp[p] = SEG_OFF[2] + capsz[p - 1, 2]
        assert Cp[p] <= CONST0
    segof = np.zeros((NP, NO), np.int64)  # segof[p-1, op] = segment offset
    for p in range(1, NP + 1):
        for s in range(NO):
            segof[p - 1, segop[p - 1, s]] = SEG_OFF[s]

    # Slot assignment + Sel matrices.
    sumC = int(Cp[1:].sum())
    selpack = np.zeros((NCORES, 128, 4 * sumC), np.float32)
    slot_prev = np.full((B, N), -1, np.int64)
    slot10 = None
    off = 0
    for p in range(1, NP + 1):
        C = int(Cp[p])
        r, n = Wr[p], Wn[p]
        o = opsc[r, n]
        bb = binof[r]
        # rank within (bin, op) group, stable order
        key = bb * NO + o
        ordx = np.argsort(key, kind="stable")
        ks = key[ordx]
        u = np.arange(len(ks), dtype=np.int64)
        if len(ks):
            first = np.r_[True, ks[1:] != ks[:-1]]
            seg_idx = np.nonzero(first)[0]
            u = u - seg_idx[np.cumsum(first) - 1]
        rank = np.empty(len(ks), np.int64)
        rank[ordx] = u
        slot = segof[p - 1, o] + rank

        # Sel matrices for this pass: for each update column, one-hot rows
        # for each child (const row for literal children, previous-pass slot
        # for op children, nothing for zero-state children).
        for side, ch in ((0, left[r, n]), (1, right[r, n])):
            lit = is_lit[r, ch]
            opa = is_opa[r, ch] & (slot_prev[r, ch] >= 0)
            rowidx = np.where(lit, CONST0 + litsc[r, ch],
                              np.where(opa, slot_prev[r, ch], -1))
            sel = np.zeros((NB, 128, C), np.float32)
            ok = rowidx >= 0
            sel[bb[ok], rowidx[ok], slot[ok]] = 1.0
            # pack: per pass block [SelL_A | SelR_A | SelL_B | SelR_B]
            for core in range(NCORES):
                a = off + side * C
                selpack[core, :, a: a + C] = sel[2 * core]
                a = off + (2 + side) * C
                selpack[core, :, a: a + C] = sel[2 * core + 1]
        slot_prev = np.full((B, N), -1, np.int64)
        slot_prev[r, n] = slot
        if p == NP:
            slot10 = (bb, slot)  # for op-root rows, aligned with Wr[NP]
        off += 4 * C

    return dict(Cp=Cp, capsz=capsz, segop=segop, sumC=sumC,
                selpack=selpack, r10=r10, slot10=slot10,
                litsc=litsc, is_lit=is_lit)


def _build_nc(Cp, capsz, segop, sumC):
    f32 = mybir.dt.float32
    C10 = int(Cp[NP])
    nc = bacc.Bacc(None)
    selpack = nc.dram_tensor("selpack", [128, 4 * sumC], f32, kind="ExternalInput")
    wmat = nc.dram_tensor("wmat", [100, NO * NI], f32, kind="ExternalInput")
    consts2 = nc.dram_tensor("consts2", [NI, 200], f32, kind="ExternalInput")
    outz = nc.dram_tensor("outz", [C10, 2 * NI], f32, kind="ExternalOutput")

    with ExitStack() as ctx:
        tc = ctx.enter_context(tile.TileContext(nc))
        singles = ctx.enter_context(tc.tile_pool(name="singles", bufs=1))
        work = ctx.enter_context(tc.tile_pool(name="work", bufs=2))
        psum = ctx.enter_context(tc.tile_pool(name="psum", bufs=1, space="PSUM"))

        w_sb = singles.tile([100, NO * NI], f32)
        nc.sync.dma_start(out=w_sb[:, :], in_=wmat[:, :])
        # Two state tiles per stream (i- and j-replication patterns).
        st = [[singles.tile([128, 100], f32, tag=f"st{k}_{s}",
                            name=f"st{k}_{s}") for k in range(2)]
              for s in range(2)]
        for s in range(2):
            for k in range(2):
                nc.vector.memset(st[s][k][:, :], 0.0)
                nc.sync.dma_start(out=st[s][k][CONST0:CONST0 + NI, :],
                                  in_=consts2[:, k * 100:(k + 1) * 100])
        selsb = singles.tile([128, 4 * sumC], f32)
        off = 0
        for p in range(1, NP + 1):
            C = int(Cp[p])
            nc.sync.dma_start(out=selsb[:, off: off + 4 * C],
                              in_=selpack[:, off: off + 4 * C])
            off += 4 * C

        outsb = singles.tile([128, 2 * NI], f32)

        off = 0
        for p in range(1, NP + 1):
            C = int(Cp[p])
            ps_l = [psum.tile([100, 128], f32, tag=f"ps_l{s}", name=f"ps_l{s}") for s in range(2)]
            ps_r = [psum.tile([100, 128], f32, tag=f"ps_r{s}", name=f"ps_r{s}") for s in range(2)]
            zt = [psum.tile([128, NI], f32, tag=f"zt{s}", name=f"zt{s}") for s in range(2)]
            sb_l = [work.tile([100, 128], f32, tag=f"sb_l{s}", name=f"sb_l{s}") for s in range(2)]
            outer = [work.tile([100, 128], f32, tag=f"outer{s}", name=f"outer{s}") for s in range(2)]
            e = [work.tile([128, NI], f32, tag=f"e{s}", name=f"e{s}") for s in range(2)]
            s_t = [work.tile([128, 1], f32, tag=f"s_t{s}", name=f"s_t{s}") for s in range(2)]
            rs = [work.tile([128, 1], f32, tag=f"rs{s}", name=f"rs{s}") for s in range(2)]

            for s in range(2):
                selL = selsb[:, off + (2 * s) * C: off + (2 * s) * C + C]
                selR = selsb[:, off + (2 * s + 1) * C: off + (2 * s + 1) * C + C]
                nc.tensor.matmul(ps_l[s][:, 0:C], st[s][0][:, :], selL,
                                 start=True, stop=True)
                nc.tensor.matmul(ps_r[s][:, 0:C], st[s][1][:, :], selR,
                                 start=True, stop=True)
                nc.scalar.copy(sb_l[s][:, 0:C], ps_l[s][:, 0:C])
            for s in range(2):
                nc.vector.tensor_mul(outer[s][:, 0:C], sb_l[s][:, 0:C],
                                     ps_r[s][:, 0:C])
            for s in range(2):
                for g in range(NO):
                    a = SEG_OFF[g]
                    w = int(capsz[p - 1, g])
                    o = int(segop[p - 1, g])
                    nc.tensor.matmul(zt[s][a:a + w, :], outer[s][:, a:a + w],
                                     w_sb[:, o * NI:(o + 1) * NI],
                                     start=True, stop=True)
            if p < NP:
                for s in range(2):
                    nc.scalar.activation(e[s][0:C, :], zt[s][0:C, :],
                                         mybir.ActivationFunctionType.Exp,
                                         accum_out=s_t[s][0:C, :])
                for s in range(2):
                    nc.vector.reciprocal_approx_fast(rs[s][0:C, :], s_t[s][0:C, :])
                    in0a = e[s][0:C, 0:NI].to_broadcast([C, NI, NI])
                    nc.vector.tensor_scalar_mul(
                        st[s][0][0:C, :].rearrange("p (i j) -> p i j", i=NI),
                        in0a, rs[s][0:C, :])
                    in0b = e[s][0:C, 0:NI].rearrange(
                        "p (o k) -> p o k", o=1).to_broadcast([C, NI, NI])
                    nc.vector.tensor_scalar_mul(
                        st[s][1][0:C, :].rearrange("p (i j) -> p i j", i=NI),
                        in0b, rs[s][0:C, :])
            else:
                for s in range(2):
                    nc.scalar.copy(outsb[0:C, s * NI:(s + 1) * NI], zt[s][0:C, :])
                nc.sync.dma_start(out=outz[:, :], in_=outsb[0:C, :])
            off += 4 * C
    nc.finalize()
    return nc


def kernel(op_table, cats, ops, lits, left, right, mask):
    global LAST_RESULTS
    op_table = np.asarray(op_table, np.float32)
    cats = np.asarray(cats)
    plan = _plan(cats, np.asarray(ops), np.asarray(lits),
                 np.asarray(left), np.asarray(right), np.asarray(mask))

    nc = _build_nc(plan["Cp"], plan["capsz"], plan["segop"], plan["sumC"])

    # wmat[i*10+j, o*10+k] = op_table[o, i, j, k]
    wmat = np.ascontiguousarray(
        op_table.transpose(1, 2, 0, 3).reshape(100, NO * NI))
    # consts2: [10, 0:100] i-replication one-hots, [10, 100:200] j-replication
    eye = np.eye(NI, dtype=np.float32)
    consts2 = np.concatenate(
        [np.repeat(eye, NI, axis=1), np.tile(eye, (1, NI))], axis=1)

    in_maps = [{"selpack": np.ascontiguousarray(plan["selpack"][c]),
                "wmat": wmat, "consts2": consts2} for c in range(NCORES)]

    res = run_bass_kernel_spmd(nc, in_maps, list(range(NCORES)), trace=TRACE)
    LAST_RESULTS = res

    # Assemble the full (B, NI) output on the host (index selection only).
    out = np.zeros((B, NI), np.float32)
    litsc, is_lit = plan["litsc"], plan["is_lit"]
    lit_rows = np.nonzero(cats[:, 0] == 0)[0]
    oh = 10.0 * np.eye(NI, dtype=np.float32)[litsc[lit_rows, 0]]
    out[lit_rows] = np.where(is_lit[lit_rows, 0][:, None], oh, 0.0)

    r10 = plan["r10"]
    bb, slot = plan["slot10"]
    for c in range(NCORES):
        z = np.asarray(res.results[c]["outz"])  # (C10, 20)
        for s in range(2):
            selmask = bb == 2 * c + s
            rows = r10[selmask]
            out[rows] = z[slot[selmask], s * NI:(s + 1) * NI]
    return out


# revision 5
# speedup vs baseline: 6.1226x; 1.0997x over previous
"""Trainium2 Bass kernel for nn_CruxMiniCircuit (gnn_message_passing).

Reference semantics: B independent rows; each row is a circuit of N nodes
(literal nodes hold a fixed one-hot distribution over 10 ints, op nodes
combine left/right child distributions through a per-op bilinear table
followed by softmax).  The reference runs 10 synchronous passes over all
nodes and returns only the root (node 0) logits per row.

Only node 0's dependency cone matters, unrolled 10 passes deep; the
per-row worklists are tiny (~5 updates/row).  The host precomputes the
integer structure; the device does all floating-point math.

The per-pass pipeline runs entirely on compute engines with NO DMA /
GPSIMD work on the critical path:

  * The per-pass state lives TRANSPOSED in SBUF as bf16 matmul weights:
    stall[u, i*10+j] = value_i(update u) (plus a j-replication copy),
    rows 96..106 hold the constant literal one-hots.
  * Child gather + replication = ONE bf16 matmul per side per pass with a
    host-built one-hot selection matrix as the moving operand
    (ps_l = st.T @ SelL builds the replicated left operand directly).
  * outer = ps_l * ps_r (one DVE PSUM->SBUF copy + one DVE mul).
  * Updates are op-sorted into fixed PE-tile segments (offsets 0/32/64),
    so the bilinear contraction is 3 static matmuls writing z transposed
    ([updates, 10]) straight into one PSUM tile.
  * softmax: one ScalarE exp with accum_out (row sums for free), one DVE
    reciprocal_approx_fast, and two DVE broadcast tensor_scalar_muls that
    write the next pass's two state slices.

Two independent row-streams per core interleave to hide cross-engine
latency.  Input selection matrices stream in over 10 pipelined DMAs (the
first from the SP queue, the rest from the otherwise-idle GPSIMD queue)
so only ~2us of DMA latency is exposed at startup.

Sharding: pure data parallel; op-root rows are greedily bin-packed onto
8 cores x 2 streams balancing per-(pass, op) update counts; literal-root
rows never touch the device.
"""

import sys
from contextlib import ExitStack

import numpy as np

sys.path.insert(0, "/opt/trn_rl_repo")

import ml_dtypes
import concourse.bass as bass  # noqa: F401
import concourse.tile as tile
from concourse import bacc, mybir
from concourse.bass_utils import run_bass_kernel_spmd

B, N = 2048, 1023
NI, NO, NP = 10, 3, 10  # n_ints, n_ops, n_passes
NCORES = 8
NB = 16  # bins = cores x streams
CONST0 = 96  # state rows 96..106 hold the literal one-hot constants
SEL_K = CONST0 + NI  # contraction depth of the selection matmuls
SEG_OFF = (0, 32, 64)  # PE-tile-aligned op segment offsets within a pass
CAPS = (32, 32, 32)
BF16 = ml_dtypes.bfloat16

TRACE = False
LAST_RESULTS = None


def _plan(cats, ops, lits, left, right, mask):
    """Integer-only host preprocessing: worklists, binning, Sel matrices."""
    left = np.clip(left.astype(np.int64), 0, N - 1)
    right = np.clip(right.astype(np.int64), 0, N - 1)
    opsc = np.clip(ops.astype(np.int64), 0, NO - 1)
    litsc = np.clip(lits.astype(np.int64), 0, NI - 1)
    m = mask.astype(bool)
    is_lit = (cats == 0) & m
    is_opa = (cats == 1) & m

    # Worklists W[p]: (row, node) updates needed at pass p.
    Wr = [None] * (NP + 1)
    Wn = [None] * (NP + 1)
    r10 = np.nonzero(cats[:, 0] == 1)[0].astype(np.int64)
    Wr[NP], Wn[NP] = r10, np.zeros(len(r10), np.int64)
    need = np.zeros((B, N), bool)
    for p in range(NP, 1, -1):
        r, n = Wr[p], Wn[p]
        cr = np.concatenate([r, r])
        cn = np.concatenate([left[r, n], right[r, n]])
        keep = is_opa[cr, cn]
        need[:] = False
        need[cr[keep], cn[keep]] = True
        rr, nn = np.nonzero(need)
        Wr[p - 1], Wn[p - 1] = rr.astype(np.int64), nn.astype(np.int64)

    # Greedy bin-packing of op-root rows onto NB bins balancing per-(pass,
    # op) update counts (each bin's per-pass op segments must fit the fixed
    # PE-tile caps).
    cnt = np.zeros((B, NP, NO), np.int64)
    for p in range(1, NP + 1):
        np.add.at(cnt, (Wr[p], p - 1, opsc[Wr[p], Wn[p]]), 1)
    tot = cnt[r10].sum(axis=(1, 2))
    order = r10[np.argsort(-tot, kind="stable")]
    bins = np.zeros((NB, NP, NO), np.int64)
    binof = np.full(B, -1, np.int64)
    nrows = np.zeros(NB, np.int64)
    for r in order:
        c = cnt[r]
        best, bestscore = -1, None
        for b in range(NB):
            new = bins[b] + c
            score = (np.sort(new.reshape(-1))[::-1].tolist(), int(nrows[b]))
            if bestscore is None or score < bestscore:
                best, bestscore = b, score
        bins[best] += c
        binof[r] = best
        nrows[best] += 1

    # Per-pass op->segment permutation: largest max-count op takes the last
    # segment (its cap bounds C_p); caps checked per pass.
    segop = np.zeros((NP, NO), np.int64)  # segop[p-1, seg] = op
    capsz = np.zeros((NP, NO), np.int64)  # actual segment sizes used
    Cp = np.zeros(NP + 1, np.int64)
    for p in range(1, NP + 1):
        mx = bins[:, p - 1, :].max(axis=0)  # per-op max over bins
        o_ord = np.argsort(mx, kind="stable")  # ascending
        segop[p - 1] = [o_ord[0], o_ord[1], o_ord[2]]
        for s in range(NO):
            assert mx[segop[p - 1, s]] <= CAPS[s], (p, s, mx)
        capsz[p - 1] = [CAPS[0], CAPS[1], mx[o_ord[2]]]
        Cp[p] = SEG_OFF[2] + capsz[p - 1, 2]
        assert Cp[p] <= CONST0
    segof = np.zeros((NP, NO), np.int64)  # segof[p-1, op] = segment offset
    for p in range(1, NP + 1):
        for s in range(NO):
            segof[p - 1, segop[p - 1, s]] = SEG_OFF[s]

    # Slot assignment + Sel matrices.
    sumC = int(Cp[1:].sum())
    selpack = np.zeros((NCORES, SEL_K, 4 * sumC), np.float32)
    slot_prev = np.full((B, N), -1, np.int64)
    slot10 = None
    off = 0
    for p in range(1, NP + 1):
        C = int(Cp[p])
        r, n = Wr[p], Wn[p]
        o = opsc[r, n]
        bb = binof[r]
        # rank within (bin, op) group, stable order
        key = bb * NO + o
        ordx = np.argsort(key, kind="stable")
        ks = key[ordx]
        u = np.arange(len(ks), dtype=np.int64)
        if len(ks):
            first = np.r_[True, ks[1:] != ks[:-1]]
            seg_idx = np.nonzero(first)[0]
            u = u - seg_idx[np.cumsum(first) - 1]
        rank = np.empty(len(ks), np.int64)
        rank[ordx] = u
        slot = segof[p - 1, o] + rank

        # Sel matrices for this pass: for each update column, one-hot rows
        # for each child (const row for literal children, previous-pass slot
        # for op children, nothing for zero-state children).
        for side, ch in ((0, left[r, n]), (1, right[r, n])):
            lit = is_lit[r, ch]
            opa = is_opa[r, ch] & (slot_prev[r, ch] >= 0)
            rowidx = np.where(lit, CONST0 + litsc[r, ch],
                              np.where(opa, slot_prev[r, ch], -1))
            sel = np.zeros((NB, SEL_K, C), np.float32)
            ok = rowidx >= 0
            sel[bb[ok], rowidx[ok], slot[ok]] = 1.0
            # pack: per pass block [SelL_A | SelR_A | SelL_B | SelR_B]
            for core in range(NCORES):
                a = off + side * C
                selpack[core, :, a: a + C] = sel[2 * core]
                a = off + (2 + side) * C
                selpack[core, :, a: a + C] = sel[2 * core + 1]
        slot_prev = np.full((B, N), -1, np.int64)
        slot_prev[r, n] = slot
        if p == NP:
            slot10 = (bb, slot)  # for op-root rows, aligned with Wr[NP]
        off += 4 * C

    return dict(Cp=Cp, capsz=capsz, segop=segop, sumC=sumC,
                selpack=selpack, r10=r10, slot10=slot10,
                litsc=litsc, is_lit=is_lit)


def _build_nc(Cp, capsz, segop, sumC):
    f32 = mybir.dt.float32
    bf = mybir.dt.bfloat16
    C10 = int(Cp[NP])
    nc = bacc.Bacc(None)
    selpack = nc.dram_tensor("selpack", [SEL_K, 4 * sumC], bf, kind="ExternalInput")
    wmat = nc.dram_tensor("wmat", [100, NO * NI], bf, kind="ExternalInput")
    consts4 = nc.dram_tensor("consts4", [NI, 400], bf, kind="ExternalInput")
    outz = nc.dram_tensor("outz", [C10, 2 * NI], f32, kind="ExternalOutput")

    with ExitStack() as ctx:
        tc = ctx.enter_context(tile.TileContext(nc))
        singles = ctx.enter_context(tc.tile_pool(name="singles", bufs=1))
        work = ctx.enter_context(tc.tile_pool(name="work", bufs=2))
        psum = ctx.enter_context(tc.tile_pool(name="psum", bufs=1, space="PSUM"))

        # One state tile: 4 x 100 columns = (stream, replication pattern).
        stall = singles.tile([128, 400], bf)
        selsb = singles.tile([SEL_K, 4 * sumC], bf)
        w_sb = singles.tile([100, NO * NI], bf)
        outsb = singles.tile([128, 2 * NI], f32)

        # Startup: pass-1 Sel + consts + wmat from the SP queue (ordered so
        # pass 1 can start ASAP); remaining Sel slices from the GPSIMD queue.
        C1 = int(Cp[1])
        nc.sync.dma_start(out=selsb[:, 0: 4 * C1], in_=selpack[:, 0: 4 * C1])
        nc.sync.dma_start(out=stall[CONST0:CONST0 + NI, :], in_=consts4[:, :])
        nc.sync.dma_start(out=w_sb[:, :], in_=wmat[:, :])
        nc.vector.memset(stall[0:CONST0, :], 0.0)
        off = 4 * C1
        for p in range(2, NP + 1):
            C = int(Cp[p])
            nc.gpsimd.dma_start(out=selsb[:, off: off + 4 * C],
                                in_=selpack[:, off: off + 4 * C])
            off += 4 * C

        def st(s, k):  # state slice for stream s, replication pattern k
            return stall[:, (2 * s + k) * 100: (2 * s + k) * 100 + 100]

        off = 0
        for p in range(1, NP + 1):
            C = int(Cp[p])
            ps_l = [psum.tile([100, 128], f32, tag=f"ps_l{s}", name=f"ps_l{s}") for s in range(2)]
            ps_r = [psum.tile([100, 128], f32, tag=f"ps_r{s}", name=f"ps_r{s}") for s in range(2)]
            zt = [psum.tile([128, NI], f32, tag=f"zt{s}", name=f"zt{s}") for s in range(2)]
            sb_l = [work.tile([100, 128], f32, tag=f"sb_l{s}", name=f"sb_l{s}") for s in range(2)]
            outer = [work.tile([100, 128], bf, tag=f"outer{s}", name=f"outer{s}") for s in range(2)]
            e = [work.tile([128, NI], f32, tag=f"e{s}", name=f"e{s}") for s in range(2)]
            s_t = [work.tile([128, 1], f32, tag=f"s_t{s}", name=f"s_t{s}") for s in range(2)]
            rs = [work.tile([128, 1], f32, tag=f"rs{s}", name=f"rs{s}") for s in range(2)]

            k0 = 0 if p > 1 else 64  # pass 1 reads only the const rows
            for s in range(2):
                selL = selsb[k0:SEL_K, off + (2 * s) * C: off + (2 * s) * C + C]
                selR = selsb[k0:SEL_K, off + (2 * s + 1) * C: off + (2 * s + 1) * C + C]
                nc.tensor.matmul(ps_l[s][:, 0:C], st(s, 0)[k0:SEL_K, :], selL,
                                 start=True, stop=True)
                nc.tensor.matmul(ps_r[s][:, 0:C], st(s, 1)[k0:SEL_K, :], selR,
                                 start=True, stop=True)
            for s in range(2):
                nc.vector.tensor_copy(sb_l[s][:, 0:C], ps_l[s][:, 0:C])
                nc.vector.tensor_mul(outer[s][:, 0:C], sb_l[s][:, 0:C],
                                     ps_r[s][:, 0:C])
            for s in range(2):
                for g in range(NO):
                    a = SEG_OFF[g]
                    w = int(capsz[p - 1, g])
                    o = int(segop[p - 1, g])
                    nc.tensor.matmul(zt[s][a:a + w, :], outer[s][:, a:a + w],
                                     w_sb[:, o * NI:(o + 1) * NI],
                                     start=True, stop=True)
            if p < NP:
                for s in range(2):
                    nc.scalar.activation(e[s][0:C, :], zt[s][0:C, :],
                                         mybir.ActivationFunctionType.Exp,
                                         accum_out=s_t[s][0:C, :])
                for s in range(2):
                    nc.vector.reciprocal_approx_fast(rs[s][0:C, :], s_t[s][0:C, :])
                    in0a = e[s][0:C, 0:NI].to_broadcast([C, NI, NI])
                    nc.vector.tensor_scalar_mul(
                        st(s, 0)[0:C, :].rearrange("p (i j) -> p i j", i=NI),
                        in0a, rs[s][0:C, :])
                    in0b = e[s][0:C, 0:NI].rearrange(
                        "p (o k) -> p o k", o=1).to_broadcast([C, NI, NI])
                    nc.vector.tensor_scalar_mul(
                        st(s, 1)[0:C, :].rearrange("p (i j) -> p i j", i=NI),
                        in0b, rs[s][0:C, :])
            else:
                for s in range(2):
                    nc.scalar.copy(outsb[0:C, s * NI:(s + 1) * NI], zt[s][0:C, :])
                    nc.sync.dma_start(out=outz[:, s * NI:(s + 1) * NI],
                                      in_=outsb[0:C, s * NI:(s + 1) * NI])
            off += 4 * C
    nc.finalize()
    return nc


def kernel(op_table, cats, ops, lits, left, right, mask):
    global LAST_RESULTS
    op_table = np.asarray(op_table, np.float32)
    cats = np.asarray(cats)
    plan = _plan(cats, np.asarray(ops), np.asarray(lits),
                 np.asarray(left), np.asarray(right), np.asarray(mask))

    nc = _build_nc(plan["Cp"], plan["capsz"], plan["segop"], plan["sumC"])

    # wmat[i*10+j, o*10+k] = op_table[o, i, j, k]
    wmat = np.ascontiguousarray(
        op_table.transpose(1, 2, 0, 3).reshape(100, NO * NI)).astype(BF16)
    # consts4: i-replication / j-replication one-hot blocks for both streams
    eye = np.eye(NI, dtype=np.float32)
    consts4 = np.tile(np.concatenate(
        [np.repeat(eye, NI, axis=1), np.tile(eye, (1, NI))], axis=1),
        (1, 2)).astype(BF16)

    in_maps = [{"selpack": plan["selpack"][c].astype(BF16),
                "wmat": wmat, "consts4": consts4} for c in range(NCORES)]

    res = run_bass_kernel_spmd(nc, in_maps, list(range(NCORES)), trace=TRACE)
    LAST_RESULTS = res

    # Assemble the full (B, NI) output on the host (index selection only).
    out = np.zeros((B, NI), np.float32)
    litsc, is_lit = plan["litsc"], plan["is_lit"]
    lit_rows = np.nonzero(cats[:, 0] == 0)[0]
    oh = 10.0 * np.eye(NI, dtype=np.float32)[litsc[lit_rows, 0]]
    out[lit_rows] = np.where(is_lit[lit_rows, 0][:, None], oh, 0.0)

    r10 = plan["r10"]
    bb, slot = plan["slot10"]
    for c in range(NCORES):
        z = np.asarray(res.results[c]["outz"])  # (C10, 20)
        for s in range(2):
            selmask = bb == 2 * c + s
            rows = r10[selmask]
            out[rows] = z[slot[selmask], s * NI:(s + 1) * NI]
    return out
